# revision 13
# baseline (speedup 1.0000x reference)
"""CommAwareGCN Trainium2 kernel (8 NeuronCores, SPMD).

Algorithm notes
---------------
The reference computes, per GCN layer, ``segment_sum(f(feats[dst]), src)``
where ``f`` is a per-row Linear(+ReLU).  Row gather commutes with per-row
ops, so this equals ``A @ f(feats)`` with ``A[s,d] = #edges(s,d)`` — the
per-edge bias inside the segment-sum is reproduced exactly because each
edge contributes ``f(feats)[dst]`` (bias included) once.  The network
collapses to:

    g   = relu(X @ W1 + b1)          # node-wise
    y1  = A @ g
    h   = y1 @ W2 + b2               # node-wise (bias pre-scatter)
    y2  = A @ h
    out = y2 @ Wfc + bfc             # node-wise

Distribution (8 cores): nodes are sharded contiguously; edges are
partitioned by the owner of ``src`` so each core's scatter-accumulate
lands only in its own shard (PSUM accumulation via one-hot matmuls —
no cross-core reduction).  The gather side reads a full replicated
bf16 node table from local HBM, built with an AllGather.  Edges are
host-sorted by src and packed into 128-edge chunks aligned to 128-row
src tiles; a chunk's scatter is ``psum[f, r] += Ggath[e, f].T-contract
OneHot[e, r]`` on the TensorEngine, with the one-hots precomputed on
the host and streamed from DRAM (they are static per-edge data, reused
identically by both sparse passes).
"""

import math
import os
import sys

import ml_dtypes
import numpy as np

# ---------------------------------------------------------------------------
# constants (hardcoded problem shape)
# ---------------------------------------------------------------------------
N_NODES = 50000
N_EDGES = 1600000
F = 128          # feature / hidden dim
NCLS = 40
N_CORES = 8
P = 128

BF16 = ml_dtypes.bfloat16


def _import_concourse():
    import concourse.bass as bass  # noqa: F401
    return bass


# ---------------------------------------------------------------------------
# host-side preprocessing: shard + sort + chunk-pack the edges
# ---------------------------------------------------------------------------
def preprocess(node_features, edge_index, W1, b1, W2, b2, Wfc, bfc,
               n_cores=N_CORES, shard=None):
    n_nodes = node_features.shape[1]
    feat = np.asarray(node_features, np.float32)[0]          # [N, F]
    src = np.asarray(edge_index[0, 0], np.int64).astype(np.int64)
    dst = np.asarray(edge_index[0, 1], np.int64).astype(np.int64)
    n_edges = src.shape[0]

    if shard is None:
        shard = int(math.ceil(n_nodes / (n_cores * P))) * P  # nodes per core
    NP = shard * n_cores
    T = shard // P                                            # tiles per core
    NT = NP // P                                              # global tiles

    # padded, bf16, feature-major node features
    xT = np.zeros((F, NP), BF16)
    xT[:, :n_nodes] = feat.T.astype(BF16)

    # Bucket edges into (src-tile, dst-half) groups.  dma_gather indices are
    # int16, so the node table is addressed as two halves of H0 rows each.
    H0 = NP // 2
    assert H0 - 1 <= 32767
    half = (dst >= H0).astype(np.int64)
    gtile = (src // P).astype(np.int64)                       # global tile id
    # dst as fastest sort key: ascending gather addresses within each call
    order = np.lexsort((dst, half, gtile))
    ss = src[order]
    ds = dst[order]
    hh = half[order]
    gt = gtile[order]
    grp = gt * 2 + hh
    counts = np.bincount(grp, minlength=NT * 2).reshape(NT, 2)
    C_lo = max(1, int(math.ceil(counts[:, 0].max() / P)))     # lo chunks/tile
    C_hi = max(1, int(math.ceil(counts[:, 1].max() / P)))     # hi chunks/tile
    C = C_lo + C_hi
    gstarts = np.concatenate([[0], np.cumsum(counts.reshape(-1))])
    within = np.arange(n_edges) - gstarts[grp]
    slot = gt * (C * P) + hh * (C_lo * P) + within

    flat_dst = np.zeros(NT * C * P, np.int16)                 # pad: gather row 0
    flat_sl = np.full(NT * C * P, -1.0, np.float32)           # pad: one-hot 0
    flat_dst[slot] = (ds - hh * H0).astype(np.int16)
    flat_sl[slot] = (ss - gt * P).astype(np.float32)

    flat_dst = flat_dst.reshape(n_cores, T * C, P)
    flat_sl = flat_sl.reshape(n_cores, T * C, P)

    def pack_idx16(tokens):
        # tokens [ncols, 128] in call-local order -> [128, ncols*8] int16
        # (16-partition wrap, replicated over the 8 partition groups)
        flat = tokens.reshape(-1)
        ncol16 = flat.shape[0] // 16
        region = flat.reshape(ncol16, 16).T                   # [16, ncol16]
        return np.tile(region, (8, 1))                        # [128, ncol16]

    per_core = []
    w1 = np.ascontiguousarray(np.asarray(W1, np.float32).astype(BF16))
    w2 = np.ascontiguousarray(np.asarray(W2, np.float32).astype(BF16))
    wfc_np = np.asarray(Wfc, np.float32).astype(BF16)         # [F, NCLS]
    b1c = np.asarray(b1, np.float32).reshape(F, 1).copy()
    b2c = np.asarray(b2, np.float32).reshape(F, 1).copy()
    bfcc = np.asarray(bfc, np.float32).reshape(-1, 1).copy()
    ident = np.eye(P, dtype=np.float32).astype(BF16)
    rr = np.arange(P, dtype=np.float32)

    for k in range(n_cores):
        # idx16: per-(tile, half) call regions, concatenated in column order
        regions = []
        fd = flat_dst[k]                                      # [T*C, P]
        for t in range(T):
            regions.append(pack_idx16(fd[t * C:t * C + C_lo]))
            regions.append(pack_idx16(fd[t * C + C_lo:(t + 1) * C]))
        idx16 = np.concatenate(regions, axis=1)               # [P, T*C*8]
        # host-precomputed scatter one-hots: oh[e, col*P + r] =
        #   (src_local[col, e] == r), pad slots (-1) give all-zero rows
        oh = (flat_sl[k][:, :, None] == rr).astype(BF16)      # [NCH, e, r]
        oh = np.ascontiguousarray(
            oh.transpose(1, 0, 2).reshape(P, T * C * P))
        per_core.append({
            "xT": np.ascontiguousarray(xT[:, k * shard:(k + 1) * shard]),
            "w1": w1, "b1": b1c, "w2": w2, "b2": b2c,
            "wfc": np.ascontiguousarray(wfc_np), "bfc": bfcc,
            "idx16": np.ascontiguousarray(idx16),
            "oh": oh, "ident": ident,
        })
    meta = dict(shard=shard, NP=NP, T=T, C=C, C_lo=C_lo, C_hi=C_hi,
                n_cores=n_cores, n_nodes=n_nodes, ncls=bfcc.shape[0],
                nqueues=int(os.environ.get("KQ", "4")),
                single_packet=bool(int(os.environ.get("KSP", "0"))))
    return per_core, meta


# ---------------------------------------------------------------------------
# device program
# ---------------------------------------------------------------------------
def build_program(meta):
    from contextlib import ExitStack

    import concourse.bacc as bacc
    import concourse.bass as bass
    import concourse.tile as tile
    from concourse import mybir

    S = meta["shard"]
    NP = meta["NP"]
    T = meta["T"]
    C = meta["C"]
    C_lo = meta["C_lo"]
    C_hi = meta["C_hi"]
    n_cores = meta["n_cores"]
    ncls = meta["ncls"]
    NCH = T * C
    H0 = NP // 2
    f32 = mybir.dt.float32
    bf16 = mybir.dt.bfloat16
    i16 = mybir.dt.int16

    # node-linear free-dim tiling
    NLIN = 448 if S % 448 == 0 else P
    J = S // NLIN

    nc = bacc.Bacc("TRN2", target_bir_lowering=False, debug=False,
                   num_devices=n_cores,
                   num_swdge_queues=meta.get("nqueues", 1))

    xT_d = nc.declare_dram_parameter("xT", [F, S], bf16, isOutput=False)
    w1_d = nc.declare_dram_parameter("w1", [F, F], bf16, isOutput=False)
    b1_d = nc.declare_dram_parameter("b1", [F, 1], f32, isOutput=False)
    w2_d = nc.declare_dram_parameter("w2", [F, F], bf16, isOutput=False)
    b2_d = nc.declare_dram_parameter("b2", [F, 1], f32, isOutput=False)
    wfc_d = nc.declare_dram_parameter("wfc", [F, ncls], bf16, isOutput=False)
    bfc_d = nc.declare_dram_parameter("bfc", [ncls, 1], f32, isOutput=False)
    idx_d = nc.declare_dram_parameter("idx16", [P, NCH * 8], i16, isOutput=False)
    oh_d = nc.declare_dram_parameter("oh", [P, NCH * P], bf16, isOutput=False)
    ident_d = nc.declare_dram_parameter("ident", [P, P], bf16, isOutput=False)
    out_d = nc.declare_dram_parameter("outT", [ncls, S], f32, isOutput=True)

    g_shard = nc.dram_tensor("g_shard", [S, F], bf16)
    g_full = nc.dram_tensor("g_full", [NP, F], bf16, addr_space="Shared")
    h_shard = nc.dram_tensor("h_shard", [S, F], bf16)
    h_full = nc.dram_tensor("h_full", [NP, F], bf16, addr_space="Shared")

    groups = [list(range(n_cores))]

    with tile.TileContext(nc) as tc, ExitStack() as ctx:
        const = ctx.enter_context(tc.tile_pool(name="const", bufs=1))
        gath = ctx.enter_context(tc.tile_pool(name="gath", bufs=3))
        ohp = ctx.enter_context(tc.tile_pool(name="ohp", bufs=3))
        trp = ctx.enter_context(tc.tile_pool(name="trp", bufs=3))
        ps_lin = ctx.enter_context(tc.tile_pool(name="ps_lin", bufs=2, space="PSUM"))
        ps_y = ctx.enter_context(tc.tile_pool(name="ps_y", bufs=2, space="PSUM"))
        ps_tr = ctx.enter_context(tc.tile_pool(name="ps_tr", bufs=2, space="PSUM"))

        # ---- persistent SBUF state -------------------------------------
        w1_sb = const.tile([F, F], bf16)
        nc.sync.dma_start(w1_sb[:], w1_d[:, :])
        w2_sb = const.tile([F, F], bf16)
        nc.sync.dma_start(w2_sb[:], w2_d[:, :])
        wfc_sb = const.tile([F, ncls], bf16)
        nc.sync.dma_start(wfc_sb[:], wfc_d[:, :])
        b1_sb = const.tile([F, 1], f32)
        nc.sync.dma_start(b1_sb[:], b1_d[:, :])
        b2_sb = const.tile([F, 1], f32)
        nc.sync.dma_start(b2_sb[:], b2_d[:, :])
        bfc_sb = const.tile([ncls, 1], f32)
        nc.sync.dma_start(bfc_sb[:], bfc_d[:, :])
        ident_sb = const.tile([P, P], bf16)
        nc.sync.dma_start(ident_sb[:], ident_d[:, :])
        idx_sb = const.tile([P, NCH * 8], i16)
        nc.sync.dma_start(idx_sb[:], idx_d[:, :])
        xT_sb = const.tile([F, S], bf16)
        nc.sync.dma_start(xT_sb[:], xT_d[:, :])

        gT_sb = const.tile([F, S], bf16)
        y1T_sb = const.tile([F, S], bf16)
        hT_sb = const.tile([F, S], bf16)
        y2T_sb = const.tile([F, S], bf16)
        out_sb = const.tile([ncls, S], f32)

        def node_linear(dst_sb, src_sb, w_sb, b_sb, func, width):
            # dst[f_out, n] = func(w.T @ src + b) per NLIN-wide node slab
            for j in range(J):
                sl = slice(j * NLIN, (j + 1) * NLIN)
                pt = ps_lin.tile([P, NLIN], mybir.dt.float32, tag="pslin")
                nc.tensor.matmul(pt[:width, :], lhsT=w_sb[:, :width],
                                 rhs=src_sb[:, sl], start=True, stop=True)
                nc.scalar.activation(dst_sb[:width, sl], pt[:width, :],
                                     func, bias=b_sb[:width, :], scale=1.0)

        def transpose_to(dram, src_sb):
            # src_sb [F, S] feature-major -> dram [S, F] node-major
            for t in range(T):
                pt = ps_tr.tile([P, P], bf16, space="PSUM", tag="pstr")
                nc.tensor.transpose(pt[:], src_sb[:, t * P:(t + 1) * P],
                                    ident_sb[:])
                st = trp.tile([P, P], bf16, tag="trst")
                nc.vector.tensor_copy(st[:], pt[:])
                nc.sync.dma_start(dram[t * P:(t + 1) * P, :], st[:])

        sparse_variant = meta.get("sparse_variant", 0)
        import itertools
        SINGLE_PACKET = meta.get("single_packet", False)
        qrr = itertools.cycle(range(meta.get("nqueues", 1)))

        def sparse_pass(table, out_sbuf):
            # out_sbuf[f, r_local] = sum over edges(src=r) of table[dst]
            for t in range(T):
                gg = gath.tile([P, C * P], bf16, tag="gg")
                gg3 = gg[:].rearrange("p (c f) -> p c f", f=P)
                ohs = ohp.tile([P, C * P], bf16, tag="oh")
                nc.sync.dma_start(ohs[:], oh_d[:, t * C * P:(t + 1) * C * P])
                base = t * C
                # per-call descriptor cap: >=1024 idxs per SWDGE call is
                # unreliable when the NEFF also carries collectives
                CAP = 7
                for half, (h_base, h_cnt, tab) in enumerate(
                        [(0, C_lo, table[0:H0, :]),
                         (C_lo, C_hi, table[H0:NP, :])]):
                    for c0 in range(0, h_cnt, CAP):
                        cn = min(CAP, h_cnt - c0)
                        b = base + h_base + c0
                        nc.gpsimd.dma_gather(
                            out_ap=gg3[:, h_base + c0:h_base + c0 + cn, :],
                            in_ap=tab,
                            idxs_ap=idx_sb[:, b * 8:(b + cn) * 8],
                            num_idxs=cn * P, num_idxs_reg=cn * P,
                            elem_size=P, single_packet=SINGLE_PACKET,
                            queue_num=next(qrr))
                if sparse_variant == 1:      # gathers only
                    nc.scalar.copy(out_sbuf[:, t * P:(t + 1) * P],
                                   gg[:, 0:P])
                    continue
                ps = ps_y.tile([P, P], mybir.dt.float32, tag="psy")
                for c in range(C):
                    nc.tensor.matmul(ps[:], lhsT=gg[:, c * P:(c + 1) * P],
                                     rhs=ohs[:, c * P:(c + 1) * P],
                                     start=(c == 0), stop=(c == C - 1))
                nc.scalar.copy(out_sbuf[:, t * P:(t + 1) * P], ps[:])

        stages = meta.get("stages", 5)
        # ---- stage A: g = relu(X @ W1 + b1)  (feature-major) -----------
        node_linear(gT_sb, xT_sb, w1_sb, b1_sb,
                    mybir.ActivationFunctionType.Relu, P)
        # ---- stage B/C: build replicated node-major g table ------------
        transpose_to(g_shard, gT_sb)
        nc.gpsimd.collective_compute(
            "AllGather", mybir.AluOpType.bypass, replica_groups=groups,
            ins=[g_shard[:, :]], outs=[g_full[:, :]])
        if stages >= 2:
            # ---- stage D: y1 = A @ g -----------------------------------
            sparse_pass(g_full, y1T_sb)
        if stages >= 3:
            # ---- stage E: h = y1 @ W2 + b2 -----------------------------
            node_linear(hT_sb, y1T_sb, w2_sb, b2_sb,
                        mybir.ActivationFunctionType.Identity, P)
            # ---- stage F: replicated h table ---------------------------
            transpose_to(h_shard, hT_sb)
            nc.gpsimd.collective_compute(
                "AllGather", mybir.AluOpType.bypass, replica_groups=groups,
                ins=[h_shard[:, :]], outs=[h_full[:, :]])
        if stages >= 4:
            # ---- stage G: y2 = A @ h -----------------------------------
            sparse_pass(h_full, y2T_sb)
        if stages >= 5:
            # ---- stage H: out = y2 @ Wfc + bfc -------------------------
            node_linear(out_sb, y2T_sb, wfc_sb, bfc_sb,
                        mybir.ActivationFunctionType.Identity, ncls)
        else:
            src_dbg = {1: gT_sb, 2: y1T_sb, 3: hT_sb, 4: y2T_sb}[stages]
            nc.scalar.copy(out_sb[:ncls, :], src_dbg[:ncls, :])
        nc.sync.dma_start(out_d[:, :], out_sb[:])

    nc.compile()
    return nc


# ---------------------------------------------------------------------------
# execution
# ---------------------------------------------------------------------------
def run(inputs, trace=False, trace_kwargs=None):
    """Returns (full_output [1, N, CLS] f32, exec_time_ns or None)."""
    from concourse.bass_utils import run_bass_kernel_spmd

    per_core, meta = preprocess(
        inputs["node_features"], inputs["edge_index"],
        inputs["W1"], inputs["b1"], inputs["W2"], inputs["b2"],
        inputs["Wfc"], inputs["bfc"])
    nc = build_program(meta)
    res = run_bass_kernel_spmd(
        nc, per_core, list(range(meta["n_cores"])),
        trace=trace, **(trace_kwargs or {}))
    outs = [res.results[k]["outT"] for k in range(meta["n_cores"])]
    full = np.concatenate(outs, axis=1).T[:meta["n_nodes"]]
    out = np.ascontiguousarray(full, dtype=np.float32)[None]
    return out, res.exec_time_ns


def kernel(**inputs) -> np.ndarray:
    out, _ = run(inputs, trace=False)
    return out



# revision 16
# speedup vs baseline: 1.2454x; 1.2454x over previous
"""CommAwareGCN Trainium2 kernel (8 NeuronCores, SPMD).

Algorithm notes
---------------
The reference computes, per GCN layer, ``segment_sum(f(feats[dst]), src)``
where ``f`` is a per-row Linear(+ReLU).  Row gather commutes with per-row
ops, so this equals ``A @ f(feats)`` with ``A[s,d] = #edges(s,d)`` — the
per-edge bias inside the segment-sum is reproduced exactly because each
edge contributes ``f(feats)[dst]`` (bias included) once.  The network
collapses to:

    g   = relu(X @ W1 + b1)          # node-wise
    y1  = A @ g
    h   = y1 @ W2 + b2               # node-wise (bias pre-scatter)
    y2  = A @ h
    out = y2 @ Wfc + bfc             # node-wise

Distribution (8 cores): nodes are sharded contiguously; edges are
partitioned by the owner of ``src`` so each core's scatter-accumulate
lands only in its own shard (PSUM accumulation via one-hot matmuls —
no cross-core reduction).  The gather side reads a full replicated
bf16 node table from local HBM, built with an AllGather.  Edges are
host-sorted by src and packed into 128-edge chunks aligned to 128-row
src tiles; a chunk's scatter is ``psum[f, r] += Ggath[e, f].T-contract
OneHot[e, r]`` on the TensorEngine, with the one-hots precomputed on
the host and streamed from DRAM (they are static per-edge data, reused
identically by both sparse passes).
"""

import math
import os
import sys

import ml_dtypes
import numpy as np

# ---------------------------------------------------------------------------
# constants (hardcoded problem shape)
# ---------------------------------------------------------------------------
N_NODES = 50000
N_EDGES = 1600000
F = 128          # feature / hidden dim
NCLS = 40
N_CORES = 8
P = 128

BF16 = ml_dtypes.bfloat16


def _import_concourse():
    import concourse.bass as bass  # noqa: F401
    return bass


# ---------------------------------------------------------------------------
# host-side preprocessing: shard + sort + chunk-pack the edges
# ---------------------------------------------------------------------------
def preprocess(node_features, edge_index, W1, b1, W2, b2, Wfc, bfc,
               n_cores=N_CORES, shard=None):
    n_nodes = node_features.shape[1]
    feat = np.asarray(node_features, np.float32)[0]          # [N, F]
    src = np.asarray(edge_index[0, 0], np.int64).astype(np.int64)
    dst = np.asarray(edge_index[0, 1], np.int64).astype(np.int64)
    n_edges = src.shape[0]

    if shard is None:
        shard = int(math.ceil(n_nodes / (n_cores * P))) * P  # nodes per core
    NP = shard * n_cores
    T = shard // P                                            # tiles per core
    NT = NP // P                                              # global tiles

    # Balance src-degree across tiles by permuting node ids (snake-LPT):
    # the padded chunk count C is set by the busiest (tile, dst-half)
    # group, so flattening per-tile edge counts trims gather/descgen pad.
    deg = np.bincount(src, minlength=NP).astype(np.int64)
    rank_ids = np.argsort(-deg, kind="stable")                # desc degree
    tile_seq = np.arange(NT)
    rounds = NP // NT                                         # = P
    tile_of_rank = np.empty(NP, np.int64)
    for r in range(rounds):
        seq = tile_seq if (r % 2 == 0) else tile_seq[::-1]
        tile_of_rank[r * NT:(r + 1) * NT] = seq
    new_of_old = np.empty(NP, np.int64)
    new_of_old[rank_ids] = tile_of_rank * P + np.arange(NP) // NT
    old_of_new = np.empty(NP, np.int64)
    old_of_new[new_of_old] = np.arange(NP)

    src = new_of_old[src]
    dst = new_of_old[dst]

    # padded, bf16, feature-major node features (in permuted order)
    featp = np.zeros((NP, F), np.float32)
    featp[new_of_old[:n_nodes]] = feat
    xT = np.ascontiguousarray(featp.T.astype(BF16))

    # Bucket edges into (src-tile, dst-half) groups.  dma_gather indices are
    # int16, so the node table is addressed as two halves of H0 rows each.
    H0 = NP // 2
    assert H0 - 1 <= 32767
    half = (dst >= H0).astype(np.int64)
    gtile = (src // P).astype(np.int64)                       # global tile id
    # dst as fastest sort key: ascending gather addresses within each call
    order = np.lexsort((dst, half, gtile))
    ss = src[order]
    ds = dst[order]
    hh = half[order]
    gt = gtile[order]
    grp = gt * 2 + hh
    counts = np.bincount(grp, minlength=NT * 2).reshape(NT, 2)
    C_lo = max(1, int(math.ceil(counts[:, 0].max() / P)))     # lo chunks/tile
    C_hi = max(1, int(math.ceil(counts[:, 1].max() / P)))     # hi chunks/tile
    C = C_lo + C_hi
    gstarts = np.concatenate([[0], np.cumsum(counts.reshape(-1))])
    within = np.arange(n_edges) - gstarts[grp]
    slot = gt * (C * P) + hh * (C_lo * P) + within

    flat_dst = np.zeros(NT * C * P, np.int16)                 # pad: gather row 0
    flat_sl = np.full(NT * C * P, -1.0, np.float32)           # pad: one-hot 0
    flat_dst[slot] = (ds - hh * H0).astype(np.int16)
    flat_sl[slot] = (ss - gt * P).astype(np.float32)

    flat_dst = flat_dst.reshape(n_cores, T * C, P)
    flat_sl = flat_sl.reshape(n_cores, T * C, P)

    def pack_idx16(tokens):
        # tokens [ncols, 128] in call-local order -> [128, ncols*8] int16
        # (16-partition wrap, replicated over the 8 partition groups)
        flat = tokens.reshape(-1)
        ncol16 = flat.shape[0] // 16
        region = flat.reshape(ncol16, 16).T                   # [16, ncol16]
        return np.tile(region, (8, 1))                        # [128, ncol16]

    per_core = []
    w1 = np.ascontiguousarray(np.asarray(W1, np.float32).astype(BF16))
    w2 = np.ascontiguousarray(np.asarray(W2, np.float32).astype(BF16))
    wfc_np = np.asarray(Wfc, np.float32).astype(BF16)         # [F, NCLS]
    b1c = np.asarray(b1, np.float32).reshape(F, 1).copy()
    b2c = np.asarray(b2, np.float32).reshape(F, 1).copy()
    bfcc = np.asarray(bfc, np.float32).reshape(-1, 1).copy()
    ident = np.eye(P, dtype=np.float32).astype(BF16)
    rr = np.arange(P, dtype=np.float32)

    for k in range(n_cores):
        # idx16: per-(tile, half) call regions, concatenated in column order
        regions = []
        fd = flat_dst[k]                                      # [T*C, P]
        for t in range(T):
            regions.append(pack_idx16(fd[t * C:t * C + C_lo]))
            regions.append(pack_idx16(fd[t * C + C_lo:(t + 1) * C]))
        idx16 = np.concatenate(regions, axis=1)               # [P, T*C*8]
        # host-precomputed scatter one-hots: oh[e, col*P + r] =
        #   (src_local[col, e] == r), pad slots (-1) give all-zero rows
        oh = (flat_sl[k][:, :, None] == rr).astype(BF16)      # [NCH, e, r]
        oh = np.ascontiguousarray(
            oh.transpose(1, 0, 2).reshape(P, T * C * P))
        per_core.append({
            "xT": np.ascontiguousarray(xT[:, k * shard:(k + 1) * shard]),
            "w1": w1, "b1": b1c, "w2": w2, "b2": b2c,
            "wfc": np.ascontiguousarray(wfc_np), "bfc": bfcc,
            "idx16": np.ascontiguousarray(idx16),
            "oh": oh, "ident": ident,
        })
    meta = dict(shard=shard, NP=NP, T=T, C=C, C_lo=C_lo, C_hi=C_hi,
                n_cores=n_cores, n_nodes=n_nodes, ncls=bfcc.shape[0],
                nqueues=int(os.environ.get("KQ", "4")),
                single_packet=bool(int(os.environ.get("KSP", "0"))),
                new_of_old=new_of_old)
    return per_core, meta


# ---------------------------------------------------------------------------
# device program
# ---------------------------------------------------------------------------
def build_program(meta):
    from contextlib import ExitStack

    import concourse.bacc as bacc
    import concourse.bass as bass
    import concourse.tile as tile
    from concourse import mybir

    S = meta["shard"]
    NP = meta["NP"]
    T = meta["T"]
    C = meta["C"]
    C_lo = meta["C_lo"]
    C_hi = meta["C_hi"]
    n_cores = meta["n_cores"]
    ncls = meta["ncls"]
    NCH = T * C
    H0 = NP // 2
    f32 = mybir.dt.float32
    bf16 = mybir.dt.bfloat16
    i16 = mybir.dt.int16

    # node-linear free-dim tiling
    NLIN = 448 if S % 448 == 0 else P
    J = S // NLIN

    nc = bacc.Bacc("TRN2", target_bir_lowering=False, debug=False,
                   num_devices=n_cores,
                   num_swdge_queues=meta.get("nqueues", 1))

    xT_d = nc.declare_dram_parameter("xT", [F, S], bf16, isOutput=False)
    w1_d = nc.declare_dram_parameter("w1", [F, F], bf16, isOutput=False)
    b1_d = nc.declare_dram_parameter("b1", [F, 1], f32, isOutput=False)
    w2_d = nc.declare_dram_parameter("w2", [F, F], bf16, isOutput=False)
    b2_d = nc.declare_dram_parameter("b2", [F, 1], f32, isOutput=False)
    wfc_d = nc.declare_dram_parameter("wfc", [F, ncls], bf16, isOutput=False)
    bfc_d = nc.declare_dram_parameter("bfc", [ncls, 1], f32, isOutput=False)
    idx_d = nc.declare_dram_parameter("idx16", [P, NCH * 8], i16, isOutput=False)
    oh_d = nc.declare_dram_parameter("oh", [P, NCH * P], bf16, isOutput=False)
    ident_d = nc.declare_dram_parameter("ident", [P, P], bf16, isOutput=False)
    out_d = nc.declare_dram_parameter("outT", [ncls, S], f32, isOutput=True)

    g_shard = nc.dram_tensor("g_shard", [S, F], bf16)
    g_full = nc.dram_tensor("g_full", [NP, F], bf16, addr_space="Shared")
    h_shard = nc.dram_tensor("h_shard", [S, F], bf16)
    h_full = nc.dram_tensor("h_full", [NP, F], bf16, addr_space="Shared")

    groups = [list(range(n_cores))]

    with tile.TileContext(nc) as tc, ExitStack() as ctx:
        const = ctx.enter_context(tc.tile_pool(name="const", bufs=1))
        gath = ctx.enter_context(tc.tile_pool(name="gath", bufs=3))
        ohp = ctx.enter_context(tc.tile_pool(name="ohp", bufs=3))
        trp = ctx.enter_context(tc.tile_pool(name="trp", bufs=3))
        ps_lin = ctx.enter_context(tc.tile_pool(name="ps_lin", bufs=2, space="PSUM"))
        ps_y = ctx.enter_context(tc.tile_pool(name="ps_y", bufs=2, space="PSUM"))
        ps_tr = ctx.enter_context(tc.tile_pool(name="ps_tr", bufs=2, space="PSUM"))

        # ---- persistent SBUF state -------------------------------------
        w1_sb = const.tile([F, F], bf16)
        nc.sync.dma_start(w1_sb[:], w1_d[:, :])
        w2_sb = const.tile([F, F], bf16)
        nc.sync.dma_start(w2_sb[:], w2_d[:, :])
        wfc_sb = const.tile([F, ncls], bf16)
        nc.sync.dma_start(wfc_sb[:], wfc_d[:, :])
        b1_sb = const.tile([F, 1], f32)
        nc.sync.dma_start(b1_sb[:], b1_d[:, :])
        b2_sb = const.tile([F, 1], f32)
        nc.sync.dma_start(b2_sb[:], b2_d[:, :])
        bfc_sb = const.tile([ncls, 1], f32)
        nc.sync.dma_start(bfc_sb[:], bfc_d[:, :])
        ident_sb = const.tile([P, P], bf16)
        nc.sync.dma_start(ident_sb[:], ident_d[:, :])
        idx_sb = const.tile([P, NCH * 8], i16)
        nc.sync.dma_start(idx_sb[:], idx_d[:, :])
        xT_sb = const.tile([F, S], bf16)
        nc.sync.dma_start(xT_sb[:], xT_d[:, :])

        gT_sb = const.tile([F, S], bf16)
        y1T_sb = const.tile([F, S], bf16)
        hT_sb = const.tile([F, S], bf16)
        y2T_sb = const.tile([F, S], bf16)
        out_sb = const.tile([ncls, S], f32)

        def node_linear(dst_sb, src_sb, w_sb, b_sb, func, width):
            # dst[f_out, n] = func(w.T @ src + b) per NLIN-wide node slab
            for j in range(J):
                sl = slice(j * NLIN, (j + 1) * NLIN)
                pt = ps_lin.tile([P, NLIN], mybir.dt.float32, tag="pslin")
                nc.tensor.matmul(pt[:width, :], lhsT=w_sb[:, :width],
                                 rhs=src_sb[:, sl], start=True, stop=True)
                nc.scalar.activation(dst_sb[:width, sl], pt[:width, :],
                                     func, bias=b_sb[:width, :], scale=1.0)

        def transpose_to(dram, src_sb):
            # src_sb [F, S] feature-major -> dram [S, F] node-major
            for t in range(T):
                pt = ps_tr.tile([P, P], bf16, space="PSUM", tag="pstr")
                nc.tensor.transpose(pt[:], src_sb[:, t * P:(t + 1) * P],
                                    ident_sb[:])
                st = trp.tile([P, P], bf16, tag="trst")
                nc.vector.tensor_copy(st[:], pt[:])
                nc.sync.dma_start(dram[t * P:(t + 1) * P, :], st[:])

        sparse_variant = meta.get("sparse_variant", 0)
        import itertools
        SINGLE_PACKET = meta.get("single_packet", False)
        qrr = itertools.cycle(range(meta.get("nqueues", 1)))

        def sparse_pass(table, out_sbuf):
            # out_sbuf[f, r_local] = sum over edges(src=r) of table[dst]
            for t in range(T):
                gg = gath.tile([P, C * P], bf16, tag="gg")
                gg3 = gg[:].rearrange("p (c f) -> p c f", f=P)
                ohs = ohp.tile([P, C * P], bf16, tag="oh")
                nc.sync.dma_start(ohs[:], oh_d[:, t * C * P:(t + 1) * C * P])
                base = t * C
                # per-call descriptor cap: >=1024 idxs per SWDGE call is
                # unreliable when the NEFF also carries collectives
                CAP = 7
                for half, (h_base, h_cnt, tab) in enumerate(
                        [(0, C_lo, table[0:H0, :]),
                         (C_lo, C_hi, table[H0:NP, :])]):
                    for c0 in range(0, h_cnt, CAP):
                        cn = min(CAP, h_cnt - c0)
                        b = base + h_base + c0
                        nc.gpsimd.dma_gather(
                            out_ap=gg3[:, h_base + c0:h_base + c0 + cn, :],
                            in_ap=tab,
                            idxs_ap=idx_sb[:, b * 8:(b + cn) * 8],
                            num_idxs=cn * P, num_idxs_reg=cn * P,
                            elem_size=P, single_packet=SINGLE_PACKET,
                            queue_num=next(qrr))
                if sparse_variant == 1:      # gathers only
                    nc.scalar.copy(out_sbuf[:, t * P:(t + 1) * P],
                                   gg[:, 0:P])
                    continue
                ps = ps_y.tile([P, P], mybir.dt.float32, tag="psy")
                for c in range(C):
                    nc.tensor.matmul(ps[:], lhsT=gg[:, c * P:(c + 1) * P],
                                     rhs=ohs[:, c * P:(c + 1) * P],
                                     start=(c == 0), stop=(c == C - 1))
                nc.scalar.copy(out_sbuf[:, t * P:(t + 1) * P], ps[:])

        stages = meta.get("stages", 5)
        # ---- stage A: g = relu(X @ W1 + b1)  (feature-major) -----------
        node_linear(gT_sb, xT_sb, w1_sb, b1_sb,
                    mybir.ActivationFunctionType.Relu, P)
        # ---- stage B/C: build replicated node-major g table ------------
        transpose_to(g_shard, gT_sb)
        nc.gpsimd.collective_compute(
            "AllGather", mybir.AluOpType.bypass, replica_groups=groups,
            ins=[g_shard[:, :]], outs=[g_full[:, :]])
        if stages >= 2:
            # ---- stage D: y1 = A @ g -----------------------------------
            sparse_pass(g_full, y1T_sb)
        if stages >= 3:
            # ---- stage E: h = y1 @ W2 + b2 -----------------------------
            node_linear(hT_sb, y1T_sb, w2_sb, b2_sb,
                        mybir.ActivationFunctionType.Identity, P)
            # ---- stage F: replicated h table ---------------------------
            transpose_to(h_shard, hT_sb)
            nc.gpsimd.collective_compute(
                "AllGather", mybir.AluOpType.bypass, replica_groups=groups,
                ins=[h_shard[:, :]], outs=[h_full[:, :]])
        if stages >= 4:
            # ---- stage G: y2 = A @ h -----------------------------------
            sparse_pass(h_full, y2T_sb)
        if stages >= 5:
            # ---- stage H: out = y2 @ Wfc + bfc -------------------------
            node_linear(out_sb, y2T_sb, wfc_sb, bfc_sb,
                        mybir.ActivationFunctionType.Identity, ncls)
        else:
            src_dbg = {1: gT_sb, 2: y1T_sb, 3: hT_sb, 4: y2T_sb}[stages]
            nc.scalar.copy(out_sb[:ncls, :], src_dbg[:ncls, :])
        nc.sync.dma_start(out_d[:, :], out_sb[:])

    nc.compile()
    return nc


# ---------------------------------------------------------------------------
# execution
# ---------------------------------------------------------------------------
def run(inputs, trace=False, trace_kwargs=None):
    """Returns (full_output [1, N, CLS] f32, exec_time_ns or None)."""
    from concourse.bass_utils import run_bass_kernel_spmd

    per_core, meta = preprocess(
        inputs["node_features"], inputs["edge_index"],
        inputs["W1"], inputs["b1"], inputs["W2"], inputs["b2"],
        inputs["Wfc"], inputs["bfc"])
    nc = build_program(meta)
    res = run_bass_kernel_spmd(
        nc, per_core, list(range(meta["n_cores"])),
        trace=trace, **(trace_kwargs or {}))
    outs = [res.results[k]["outT"] for k in range(meta["n_cores"])]
    full_new = np.concatenate(outs, axis=1).T           # [NP, ncls], new ids
    full = full_new[meta["new_of_old"][:meta["n_nodes"]]]
    out = np.ascontiguousarray(full, dtype=np.float32)[None]
    return out, res.exec_time_ns


def kernel(**inputs) -> np.ndarray:
    out, _ = run(inputs, trace=False)
    return out

